# revision 4
# baseline (speedup 1.0000x reference)
"""Trainium2 Bass kernel for the 1x1-conv attention block + groupnorm-swish.

Reference computation (B=2, C=128, spatial 16^3 -> N=4096):
    q = wq@query + bq; k = wk@key + bk; v = wv@value + bv   (per batch, [C, N])
    S[i, j] = sum_c q[c,i] k[c,j]; P = softmax_j(S)
    h[c, i] = sum_j v[c,j] P[i,j]
    x = wo@h + bo + value
    out = silu(group_norm(x) * gamma + beta)   (G=32 groups of 4 channels)

Sharding: 8 cores = 2 batches x 4 query-token chunks of 1024 (sequence
parallel). Each core computes k/v projections for its full batch (replicated
within the batch group), its S^T/softmax/PV chunk, and the group-norm partial
sums; a tiny [32, 2] AllReduce over each batch's 4-core group produces the
full-batch group statistics.

Layout trick: scores are computed TRANSPOSED (S^T[j, i] = k_tile^T @ q) so the
PV contraction over j needs no transposes of the softmax output. Softmax max-
subtraction is skipped (logits are ~N(0, 14); |S| < ~30 is safe in fp32).
The softmax denominator is accumulated from the exp tiles on DVE/GpSimd and
applied AFTER the output projection (column scaling commutes with the
channel-mixing matmul). The v-projection bias is folded into an effective
output bias on the host: bo_eff = wo@bv + bo.

Matmuls run in float32r (full-rate fp32 PE mode, ~1e-4 matmul error); the
v^T projection runs in bf16 (its error averages out through the softmax
weighting).
"""

import sys
import types

import ml_dtypes
import numpy as np

# The axon NTFF-profile hook module is absent from this image's antenv
# package; concourse imports it unconditionally when tracing. Install a
# functional shim (used by the test harness; harmless otherwise).
try:
    import antenv.axon_hooks  # noqa: F401
except ImportError:
    import antenv

    _mod = types.ModuleType("antenv.axon_hooks")
    _hook_box = [None]
    _mod.set_axon_ntff_profile_hook = lambda h: _hook_box.__setitem__(0, h)
    _mod.get_axon_ntff_profile_hook = lambda: _hook_box[0]
    sys.modules["antenv.axon_hooks"] = _mod
    antenv.axon_hooks = _mod
    try:
        from trn_agent_boot.trn_boot import _ntff_profile_via_ctypes

        _mod.set_axon_ntff_profile_hook(
            _ntff_profile_via_ctypes("/opt/axon/libaxon_pjrt.so")
        )
    except Exception:
        pass

import concourse.tile as tile
from concourse import bacc, mybir
from concourse.bass_utils import run_bass_kernel_spmd

B = 2
C = 128
N = 4096
NCORES = 8
CHUNKS = 4  # query-token chunks per batch
NC = N // CHUNKS  # 1024 tokens per core
JT = N // 128  # 32 key tiles of 128
G = 32  # groupnorm groups
EPS = 1e-5
GROUP_ELEMS = float((C // G) * N)  # 16384

R = mybir.dt.float32r
F32 = mybir.dt.float32
BF16 = mybir.dt.bfloat16
AF = mybir.ActivationFunctionType
ALU = mybir.AluOpType

_NC_CACHE = None


def _build():
    nc = bacc.Bacc("TRN2", target_bir_lowering=False, debug=False, num_devices=NCORES)

    q_in = nc.dram_tensor("q_in", [C, NC], R, kind="ExternalInput")
    k_in = nc.dram_tensor("k_in", [C, N], R, kind="ExternalInput")
    v_in = nc.dram_tensor("v_in", [C, N], BF16, kind="ExternalInput")
    vres_in = nc.dram_tensor("vres", [C, NC], F32, kind="ExternalInput")
    wqT_in = nc.dram_tensor("wqT", [C, C], R, kind="ExternalInput")
    wkT_in = nc.dram_tensor("wkT", [C, C], R, kind="ExternalInput")
    wvT_in = nc.dram_tensor("wvT", [C, C], BF16, kind="ExternalInput")
    woT_in = nc.dram_tensor("woT", [C, C], R, kind="ExternalInput")
    bq_in = nc.dram_tensor("bq", [C, 1], F32, kind="ExternalInput")
    bk_in = nc.dram_tensor("bk", [C, 1], F32, kind="ExternalInput")
    boe_in = nc.dram_tensor("bo_eff", [C, 1], F32, kind="ExternalInput")
    gamma_in = nc.dram_tensor("gamma", [C, 1], F32, kind="ExternalInput")
    beta_in = nc.dram_tensor("beta", [C, 1], F32, kind="ExternalInput")
    y_out = nc.dram_tensor("y_out", [C, NC], F32, kind="ExternalOutput")

    ones_np = np.ones((C, C), dtype=np.float32)
    e_np = np.zeros((C, G), dtype=np.float32)
    for c in range(C):
        e_np[c, c // (C // G)] = 1.0
    ones_dram = nc.inline_tensor(ones_np, name="ones128")
    e_dram = nc.inline_tensor(e_np, name="egrp")
    et_dram = nc.inline_tensor(np.ascontiguousarray(e_np.T), name="egrpT")

    with tile.TileContext(nc) as tc:
        with (
            tc.tile_pool(name="const", bufs=1) as const,
            tc.tile_pool(name="big", bufs=1) as big,
            tc.tile_pool(name="expp", bufs=4) as expp,
            tc.tile_pool(name="ps_st", bufs=2, space="PSUM") as ps_st,
            tc.tile_pool(name="ps_h", bufs=1, space="PSUM") as ps_h,
            tc.tile_pool(name="ps_sm", bufs=2, space="PSUM") as ps_sm,
            tc.tile_pool(name="dram", bufs=2, space="DRAM") as dram,
        ):
            # ---- constants / weights ----
            wqT = const.tile([C, C], R)
            wkT = const.tile([C, C], R)
            wvT = const.tile([C, C], BF16)
            woT = const.tile([C, C], R)
            ones_sb = const.tile([C, C], F32)
            e_sb = const.tile([C, G], F32)
            et_sb = const.tile([G, C], F32)
            bq_sb = const.tile([C, 1], F32)
            bk_sb = const.tile([C, 1], F32)
            boe_sb = const.tile([C, 1], F32)
            gamma_sb = const.tile([C, 1], F32)
            beta_sb = const.tile([C, 1], F32)
            eps_sb = const.tile([G, 1], F32)
            nc.sync.dma_start(wqT[:], wqT_in[:])
            nc.sync.dma_start(wkT[:], wkT_in[:])
            nc.sync.dma_start(wvT[:], wvT_in[:])
            nc.sync.dma_start(woT[:], woT_in[:])
            nc.sync.dma_start(ones_sb[:], ones_dram[:])
            nc.sync.dma_start(e_sb[:], e_dram[:])
            nc.sync.dma_start(et_sb[:], et_dram[:])
            nc.sync.dma_start(bq_sb[:], bq_in[:])
            nc.sync.dma_start(bk_sb[:], bk_in[:])
            nc.sync.dma_start(boe_sb[:], boe_in[:])
            nc.sync.dma_start(gamma_sb[:], gamma_in[:])
            nc.sync.dma_start(beta_sb[:], beta_in[:])
            nc.vector.memset(eps_sb[:], EPS)

            # ---- q projection: q_sb = wq @ query_chunk + bq ----
            q_raw = big.tile([C, NC], R)
            nc.sync.dma_start(q_raw[:], q_in[:])
            q_sb = big.tile([C, NC], R)
            for h in range(NC // 512):
                qp = ps_sm.tile([C, 512], F32, tag="sm")
                nc.tensor.matmul(
                    qp[:], wqT[:], q_raw[:, h * 512 : (h + 1) * 512],
                    start=True, stop=True,
                )
                nc.vector.tensor_scalar(
                    out=q_sb[:, h * 512 : (h + 1) * 512], in0=qp[:],
                    scalar1=bq_sb[:], scalar2=None, op0=ALU.add,
                )

            # ---- k projection (full batch): k_sb = wk @ key + bk ----
            k_raw = big.tile([C, N], R)
            k_sb = big.tile([C, N], R)
            for h in range(N // 512):
                sl = slice(h * 512, (h + 1) * 512)
                nc.sync.dma_start(k_raw[:, sl], k_in[:, sl])
                kp = ps_sm.tile([C, 512], F32, tag="sm")
                nc.tensor.matmul(kp[:], wkT[:], k_raw[:, sl], start=True, stop=True)
                nc.vector.tensor_scalar(
                    out=k_sb[:, sl], in0=kp[:],
                    scalar1=bk_sb[:], scalar2=None, op0=ALU.add,
                )

            # ---- v^T (unbiased, bf16): vT[j, c] = sum_c' value[c', j] wv[c, c'] ----
            v_raw = big.tile([C, N], BF16)
            nc.sync.dma_start(v_raw[:], v_in[:])
            v_raw3 = v_raw[:].rearrange("c (t j) -> c t j", j=128)
            vt_sb = big.tile([128, JT, C], R)
            for t in range(JT):
                vp = ps_sm.tile([128, C], F32, tag="sm")
                nc.tensor.matmul(vp[:], v_raw3[:, t, :], wvT[:], start=True, stop=True)
                nc.scalar.activation(out=vt_sb[:, t, :], in_=vp[:], func=AF.Copy)

            # ---- residual + effective output bias ----
            vres_sb = big.tile([C, NC], F32)
            nc.sync.dma_start(vres_sb[:], vres_in[:])
            r_sb = big.tile([C, NC], F32)
            nc.vector.tensor_scalar(
                out=r_sb[:], in0=vres_sb[:],
                scalar1=boe_sb[:], scalar2=None, op0=ALU.add,
            )

            # ---- main attention loop over 32 key tiles ----
            k_sb3 = k_sb[:].rearrange("c (t j) -> c t j", j=128)
            h_ps = ps_h.tile([C, NC], F32)
            acc_a = big.tile([128, NC], F32)
            acc_b = big.tile([128, NC], F32)
            for t in range(JT):
                st_ps = ps_st.tile([128, NC], F32, tag="st")
                for h in range(NC // 512):
                    sl = slice(h * 512, (h + 1) * 512)
                    nc.tensor.matmul(
                        st_ps[:, sl], k_sb3[:, t, :], q_sb[:, sl],
                        start=True, stop=True,
                    )
                exp_t = expp.tile([128, NC], R, tag="exp")
                nc.scalar.activation(out=exp_t[:], in_=st_ps[:], func=AF.Exp)
                for h in range(NC // 512):
                    sl = slice(h * 512, (h + 1) * 512)
                    nc.tensor.matmul(
                        h_ps[:, sl], vt_sb[:, t, :], exp_t[:, sl],
                        start=(t == 0), stop=(t == JT - 1), skip_group_check=True,
                    )
                exp_f = exp_t[:].bitcast(F32)
                if t == 0:
                    nc.vector.tensor_copy(acc_a[:], exp_f)
                elif t == 1:
                    nc.gpsimd.tensor_copy(acc_b[:], exp_f)
                elif t % 2 == 0:
                    nc.vector.tensor_add(acc_a[:], acc_a[:], exp_f)
                else:
                    nc.gpsimd.tensor_add(acc_b[:], acc_b[:], exp_f)

            # ---- softmax denominator, broadcast to all partitions ----
            nc.vector.tensor_add(acc_a[:], acc_a[:], acc_b[:])
            db_ps = ps_st.tile([C, NC], F32, tag="st")
            for h in range(NC // 512):
                sl = slice(h * 512, (h + 1) * 512)
                nc.tensor.matmul(
                    db_ps[:, sl], ones_sb[:], acc_a[:, sl], start=True, stop=True
                )
            dinv_sb = big.tile([C, NC], F32)
            nc.vector.reciprocal(dinv_sb[:], db_ps[:])

            # ---- output projection; x = o * dinv + (vres + bo_eff) ----
            h_sb = big.tile([C, NC], R)
            nc.scalar.activation(out=h_sb[:], in_=h_ps[:], func=AF.Copy)
            o_ps = ps_st.tile([C, NC], F32, tag="st")
            for h in range(NC // 512):
                sl = slice(h * 512, (h + 1) * 512)
                nc.tensor.matmul(o_ps[:, sl], woT[:], h_sb[:, sl], start=True, stop=True)
            x_sb = big.tile([C, NC], F32)
            nc.vector.tensor_mul(x_sb[:], o_ps[:], dinv_sb[:])
            nc.vector.tensor_add(x_sb[:], x_sb[:], r_sb[:])

            # ---- groupnorm partial stats: per-channel sum and sum-of-squares ----
            rowstats = big.tile([C, 2], F32)
            nc.vector.reduce_sum(rowstats[:, 0:1], x_sb[:], axis=mybir.AxisListType.X)
            xsq_sb = big.tile([C, NC], F32)
            nc.scalar.activation(
                out=xsq_sb[:], in_=x_sb[:], func=AF.Square,
                accum_out=rowstats[:, 1:2],
            )
            gs_ps = ps_sm.tile([G, 2], F32, tag="sm")
            nc.tensor.matmul(gs_ps[:], e_sb[:], rowstats[:], start=True, stop=True)
            gs_sb = big.tile([G, 2], F32)
            nc.scalar.activation(out=gs_sb[:], in_=gs_ps[:], func=AF.Copy)

            # ---- AllReduce partial stats within each batch's 4-core group ----
            cc_in = dram.tile([G, 2], F32)
            cc_out = dram.tile([G, 2], F32)
            nc.sync.dma_start(cc_in[:], gs_sb[:])
            nc.gpsimd.collective_compute(
                "AllReduce",
                ALU.add,
                replica_groups=[[0, 1, 2, 3], [4, 5, 6, 7]],
                ins=[cc_in.opt()],
                outs=[cc_out.opt()],
            )
            ar_sb = big.tile([G, 2], F32)
            nc.sync.dma_start(ar_sb[:], cc_out[:])

            # ---- group mean / rstd -> per-channel scale+bias ----
            msr = big.tile([G, 2], F32)  # [mean, rstd]
            nc.scalar.mul(msr[:, 0:1], ar_sb[:, 0:1], 1.0 / GROUP_ELEMS)
            ex2 = big.tile([G, 1], F32)
            nc.scalar.mul(ex2[:], ar_sb[:, 1:2], 1.0 / GROUP_ELEMS)
            m2 = big.tile([G, 1], F32)
            nc.vector.tensor_mul(m2[:], msr[:, 0:1], msr[:, 0:1])
            var = big.tile([G, 1], F32)
            nc.vector.tensor_sub(var[:], ex2[:], m2[:])
            sd = big.tile([G, 1], F32)
            nc.scalar.activation(
                out=sd[:], in_=var[:], func=AF.Sqrt, bias=eps_sb[:], scale=1.0
            )
            nc.vector.reciprocal(msr[:, 1:2], sd[:])
            exp_ps = ps_sm.tile([C, 2], F32, tag="sm")
            nc.tensor.matmul(exp_ps[:], et_sb[:], msr[:], start=True, stop=True)
            mr_sb = big.tile([C, 2], F32)
            nc.scalar.activation(out=mr_sb[:], in_=exp_ps[:], func=AF.Copy)
            fs_sb = big.tile([C, 1], F32)
            nc.vector.tensor_mul(fs_sb[:], mr_sb[:, 1:2], gamma_sb[:])
            fb_sb = big.tile([C, 1], F32)
            nc.vector.tensor_mul(fb_sb[:], mr_sb[:, 0:1], fs_sb[:])
            nc.vector.tensor_sub(fb_sb[:], beta_sb[:], fb_sb[:])

            # ---- out = silu(fs * x + fb) ----
            y_sb = big.tile([C, NC], F32)
            nc.scalar.activation(
                out=y_sb[:], in_=x_sb[:], func=AF.Silu, bias=fb_sb[:], scale=fs_sb[:]
            )
            nc.sync.dma_start(y_out[:], y_sb[:])

    nc.compile()
    return nc


def _get_nc():
    global _NC_CACHE
    if _NC_CACHE is None:
        _NC_CACHE = _build()
    return _NC_CACHE


def kernel(query, key, value, wq, bq, wk, bk, wv, bv, wo, bo, gamma, beta):
    nc = _get_nc()

    f32 = lambda a: np.ascontiguousarray(np.asarray(a, dtype=np.float32))
    q = f32(query).reshape(B, C, N)
    k = f32(key).reshape(B, C, N)
    v = f32(value).reshape(B, C, N)
    wq, wk, wv, wo = f32(wq), f32(wk), f32(wv), f32(wo)
    bo_eff = (wo @ f32(bv).reshape(C) + f32(bo).reshape(C)).astype(np.float32)

    shared = {
        "wqT": np.ascontiguousarray(wq.T),
        "wkT": np.ascontiguousarray(wk.T),
        "wvT": np.ascontiguousarray(wv.T).astype(ml_dtypes.bfloat16),
        "woT": np.ascontiguousarray(wo.T),
        "bq": f32(bq).reshape(C, 1),
        "bk": f32(bk).reshape(C, 1),
        "bo_eff": bo_eff.reshape(C, 1),
        "gamma": f32(gamma).reshape(C, 1),
        "beta": f32(beta).reshape(C, 1),
    }
    in_maps = []
    for p in range(NCORES):
        b, ch = divmod(p, CHUNKS)
        sl = slice(ch * NC, (ch + 1) * NC)
        in_maps.append(
            {
                "q_in": np.ascontiguousarray(q[b][:, sl]),
                "k_in": k[b],
                "v_in": v[b].astype(ml_dtypes.bfloat16),
                "vres": np.ascontiguousarray(v[b][:, sl]),
                **shared,
            }
        )

    res = run_bass_kernel_spmd(nc, in_maps, list(range(NCORES)))

    out = np.empty((B, C, N), dtype=np.float32)
    for p in range(NCORES):
        b, ch = divmod(p, CHUNKS)
        out[b][:, ch * NC : (ch + 1) * NC] = res.results[p]["y_out"]
    return out.reshape(B, C, 16, 16, 16)
